# revision 6
# baseline (speedup 1.0000x reference)
"""Bass kernel builder for nn_MixtureOfMambaBlock — 8-core SPMD.

Sharding: tokens 8-way (512/core + 128 halo for conv+scan warmup); mixer fully
local per core (weights replicated, bf16 matmuls; gate-logit path kept f32).
Post-mixer h2 all-gathered (bf16), MoE expert-parallel (one expert per core,
dense over the 2048-token half), weighted partials reduce-scattered back.
"""
import numpy as np
import concourse.bass as bass
import concourse.bacc as bacc
import concourse.mybir as mybir
import concourse.tile as tile

FP = mybir.dt.float32
FR = mybir.dt.float32r
BF = mybir.dt.bfloat16
AF = mybir.ActivationFunctionType
ALU = mybir.AluOpType

B, T, D = 2, 2048, 1024
S, INNER = 64, 2048
E, HH = 4, 2048          # experts, hid-half width
OWN, HALO = 512, 128
NH = OWN + HALO          # 640
KB = D // 128            # 8  d-blocks
MB = INNER // 128        # 16 inner-blocks
OTB = OWN // 128         # 4  own-token blocks
N_CORES = 8

INPUT_SPECS = {
    "x_sh": ([NH, D], FP),
    "ipw": ([D, 2 * INNER], BF), "ipb": ([2 * INNER], FP),
    "cw": ([INNER, 3], FP), "cb": ([INNER], FP),
    "dbw": ([INNER, 128], FR),  # dt_w || bp_w stacked on output dim
    "dtb": ([S], FP), "bpb": ([S], FP),
    "cpw": ([INNER, S], FR), "cpb": ([S], FP),
    "s2iw": ([S, INNER], BF), "s2ib": ([INNER], FP),
    "Dp": ([INNER], FP),
    "ow": ([INNER, D], BF), "ob": ([D], BF),
    "gw": ([D, E], FP), "gb": ([E], FR),
    "ew1": ([D, 2 * HH], BF), "eb1": ([2 * HH], FP),
    "ew2": ([2 * HH, D], BF), "eb2h": ([D], BF),
    "esel": ([128, E], FP),
    "rmask": ([128, 4], FP),
    "ident": ([128, 128], FP),
    "ones1": ([1, 128], FR),
    "ones1b": ([1, 128], BF),
}


def build(debug_outputs=False):
    nc = bacc.Bacc("TRN2", target_bir_lowering=False, debug=False,
                   num_devices=N_CORES)
    dp = {}
    for name, (shape, dt) in INPUT_SPECS.items():
        dp[name] = nc.dram_tensor(name, shape, dt, kind="ExternalInput")
    out_d = nc.dram_tensor("out", [OWN, D], FP, kind="ExternalOutput")

    rg = [[0, 2, 4, 6], [1, 3, 5, 7]]

    with tile.TileContext(nc) as tc:
        with (
            tc.tile_pool(name="outer", bufs=1) as po,
            tc.tile_pool(name="dram", bufs=1, space="DRAM") as pdram,
        ):
            # ---------- DRAM bounce buffers for collectives ----------
            gth_in = [pdram.tile([D, 128], BF, name=f"gth_in{t_}") for t_ in range(OTB)]
            gth_out = [pdram.tile([4 * D, 128], BF, name=f"gth_out{t_}")
                       for t_ in range(OTB)]
            gtw_in = pdram.tile([OWN, E], FP)
            gtw_out = pdram.tile([4 * OWN, E], FP)
            rs_in = [pdram.tile([OWN, D], FP, name=f"rs_in{r}") for r in range(4)]
            rs_out = [pdram.tile([128, D], FP, name=f"rs_out{r}") for r in range(4)]

            # ---------- constants / small weights ----------
            ident = po.tile([128, 128], FP)
            nc.sync.dma_start(ident[:], dp["ident"][:])

            def load_pcol(name, n, blocks):  # [n*128] -> [128, blocks] (col b = block b)
                t = po.tile([128, blocks], FP, name=f"{name}_sb")
                nc.sync.dma_start(
                    t[:], dp[name].ap().rearrange("(m p) -> p m", p=128))
                return t

            def load_vec1(name, n):  # [n] -> [n, 1]
                t = po.tile([n, 1], FP, name=f"{name}_sb")
                nc.sync.dma_start(t[:], dp[name].ap().rearrange("(s o) -> s o", o=1))
                return t

            def load_row(name, n, dt_=FP):  # [n] -> [1, n]
                t = po.tile([1, n], dt_, name=f"{name}_sb")
                nc.sync.dma_start(t[:], dp[name].ap().rearrange("(o s) -> o s", o=1))
                return t

            ones1 = po.tile([1, 128], FR)
            nc.sync.dma_start(ones1[:], dp["ones1"][:])
            ones1b = po.tile([1, 128], BF)
            nc.sync.dma_start(ones1b[:], dp["ones1b"][:])

            # persistent activations (live into MoE phase)
            xo = [po.tile([128, D], FP, name=f"xo{t_}", tag=f"xo{t_}") for t_ in range(OTB)]
            xmid = [po.tile([128, D], FP, name=f"xmid{t_}", tag=f"xmid{t_}") for t_ in range(OTB)]
            h2own = [po.tile([128, OWN], BF, name=f"h2own{kb}", tag=f"h2own{kb}")
                     for kb in range(KB)]
            wv_sb = [po.tile([128, E], FP, name=f"wv{t_}", tag=f"wv{t_}") for t_ in range(OTB)]

            # =======================================================
            # MIXER
            # =======================================================
            with (
                tc.tile_pool(name="mixer", bufs=1) as pm,
                tc.tile_pool(name="mixt", bufs=1) as pt_pool,
            ):
                hT = [pm.tile([128, NH], BF, name=f"hT{kb}", tag=f"hT{kb}") for kb in range(KB)]
                xm = [pm.tile([128, NH], FR, name=f"xm{m}", tag=f"xm{m}") for m in range(MB)]

                # ---- rmsnorm1 + transpose to hT (bf16) ----
                with nc.named_scope("rms1"), tc.tile_pool(name="ps1", bufs=1, space="PSUM") as psA:
                    for tb in range(NH // 128):
                        if tb == 0:
                            xt = pt_pool.tile([128, D], FP, tag="xt", bufs=2)
                        else:
                            xt = xo[tb - 1]
                        nc.sync.dma_start(xt[:], dp["x_sh"][tb * 128:(tb + 1) * 128, :])
                        scr = pt_pool.tile([128, D], FP, tag="scr", bufs=2)
                        sq = pt_pool.tile([128, 1], FP, tag="sq", bufs=2)
                        nc.scalar.activation(scr[:], xt[:], AF.Square, accum_out=sq[:])
                        nr = pt_pool.tile([128, 1], FP, tag="nr", bufs=2)
                        nc.vector.tensor_scalar(nr[:], sq[:], 1.0 / D, 1e-6, ALU.mult, ALU.add)
                        nc.scalar.sqrt(nr[:], nr[:])
                        nc.vector.reciprocal(nr[:], nr[:])
                        h_t = pt_pool.tile([128, D], FP, tag="scr", bufs=2)
                        nc.vector.tensor_scalar(h_t[:], xt[:], nr[:], None, ALU.mult)
                        for kb in range(KB):
                            ptr = psA.tile([128, 128], FP, tag="ptr", bufs=2)
                            nc.tensor.transpose(ptr[:], h_t[:, kb * 128:(kb + 1) * 128], ident[:])
                            nc.vector.tensor_copy(hT[kb][:, tb * 128:(tb + 1) * 128], ptr[:])

                ipb_sb = load_pcol("ipb", 2 * INNER, 32)
                cb_sb = load_pcol("cb", INNER, 16)
                cw_sb = po.tile([128, 16, 3], FP)  # [p, m, k]
                nc.sync.dma_start(cw_sb[:], dp["cw"].ap().rearrange("(m p) k -> p m k", p=128))

                # ---- in_proj (x_main half) + conv + silu ----
                with nc.named_scope("in_proj"), tc.tile_pool(name="ps2", bufs=1, space="PSUM") as psA:
                    for q in range(4):
                        wq = pt_pool.tile([128, KB, 512], BF, tag="wslab", bufs=2,
                                          name=f"wip{q}")
                        for kb in range(KB):
                            nc.gpsimd.dma_start(
                                wq[:, kb, :], dp["ipw"][kb * 128:(kb + 1) * 128,
                                                        q * 512:(q + 1) * 512])
                        for mi in range(4):
                            m = q * 4 + mi
                            xzp = pt_pool.tile([128, NH + 2], FP, tag="xzp", bufs=2)
                            nc.vector.memset(xzp[:, 0:2], 0.0)
                            for n0, nw in ((0, 512), (512, 128)):
                                px = psA.tile([128, 512], FP, tag="px", bufs=2)
                                for kb in range(KB):
                                    nc.tensor.matmul(px[:, 0:nw],
                                                     wq[:, kb, mi * 128:(mi + 1) * 128],
                                                     hT[kb][:, n0:n0 + nw],
                                                     start=(kb == 0), stop=(kb == KB - 1))
                                nc.scalar.activation(xzp[:, 2 + n0:2 + n0 + nw], px[:, 0:nw],
                                                     AF.Identity, bias=ipb_sb[:, m:m + 1])
                            cv = pt_pool.tile([128, NH], FP, tag="cv", bufs=2)
                            nc.vector.tensor_scalar(cv[:], xzp[:, 0:NH], cw_sb[:, m, 0:1],
                                                    None, ALU.mult)
                            nc.vector.scalar_tensor_tensor(cv[:], xzp[:, 1:1 + NH],
                                                           cw_sb[:, m, 1:2], cv[:],
                                                           ALU.mult, ALU.add)
                            nc.vector.scalar_tensor_tensor(cv[:], xzp[:, 2:2 + NH],
                                                           cw_sb[:, m, 2:3], cv[:],
                                                           ALU.mult, ALU.add)
                            sgc = pt_pool.tile([128, NH], FP, tag="sgc", bufs=2)
                            nc.scalar.activation(sgc[:], cv[:], AF.Sigmoid, bias=cb_sb[:, m:m + 1])
                            nc.vector.scalar_tensor_tensor(xm[m][:], cv[:], cb_sb[:, m:m + 1],
                                                           sgc[:], ALU.add, ALU.mult)

                dtb_sb = load_vec1("dtb", S)
                bpb_sb = load_vec1("bpb", S)
                cpb_sb = load_vec1("cpb", S)
                dbw_sb = pm.tile([128, MB, 128], FR, name="dbw_sb")
                nc.sync.dma_start(dbw_sb[:], dp["dbw"].ap().rearrange("(kb p) s -> p kb s", p=128))
                cpw_sb = pm.tile([128, MB, S], FR, name="cpw_sb")
                nc.sync.dma_start(cpw_sb[:], dp["cpw"].ap().rearrange("(kb p) s -> p kb s", p=128))

                # ---- dt/B/C projections + scan ----
                with nc.named_scope("scan"), tc.tile_pool(name="ps3", bufs=1, space="PSUM") as psA:
                    dt_t = pt_pool.tile([S, NH], FP, tag="dt")
                    a_t = pt_pool.tile([S, NH], FP, tag="a")
                    b_t = pt_pool.tile([S, NH], FP, tag="b")
                    c_t = pt_pool.tile([S, NH], FP, tag="c")
                    for n0, nw in ((0, 320), (320, 320)):
                        pzdb = psA.tile([128, 320], FP, tag="pzdb", bufs=2)
                        for kb in range(MB):
                            nc.tensor.matmul(pzdb[:, 0:nw], dbw_sb[:, kb, :],
                                             xm[kb][:, n0:n0 + nw],
                                             start=(kb == 0), stop=(kb == MB - 1))
                        nc.scalar.activation(dt_t[:, n0:n0 + nw], pzdb[0:S, 0:nw],
                                             AF.Sigmoid, bias=dtb_sb[:])
                        nc.vector.scalar_tensor_tensor(b_t[:, n0:n0 + nw], pzdb[S:128, 0:nw],
                                                       bpb_sb[:], dt_t[:, n0:n0 + nw],
                                                       ALU.add, ALU.mult)
                        pzc = psA.tile([S, 320], FP, tag="pzc", bufs=2)
                        for kb in range(MB):
                            nc.tensor.matmul(pzc[:, 0:nw], cpw_sb[:, kb, :],
                                             xm[kb][:, n0:n0 + nw],
                                             start=(kb == 0), stop=(kb == MB - 1))
                        nc.scalar.activation(c_t[:, n0:n0 + nw], pzc[:, 0:nw], AF.Identity,
                                             bias=cpb_sb[:])
                    nc.scalar.activation(a_t[:], dt_t[:], AF.Identity, bias=1.0, scale=-1.0)
                    st_t = pt_pool.tile([S, NH], FP, tag="st")
                    nc.vector.tensor_tensor_scan(st_t[:], a_t[:], b_t[:], 0.0,
                                                 ALU.mult, ALU.add)
                    y_t = pt_pool.tile([S, OWN], FP, tag="dt", name="y_t")
                    nc.vector.tensor_mul(y_t[:], c_t[:, HALO:NH], st_t[:, HALO:NH])

                # ---- layernorm over S (transpose - LN - transpose back) ----
                with nc.named_scope("ln"), tc.tile_pool(name="ps4", bufs=1, space="PSUM") as psA:
                    yln = pt_pool.tile([S, OWN], BF, tag="a", name="yln")
                    for i in range(OTB):
                        ptr = psA.tile([128, 128], FP, tag="ptr", bufs=2)
                        nc.tensor.transpose(ptr[:, 0:S], y_t[:, i * 128:(i + 1) * 128],
                                            ident[0:S, 0:S])
                        yT = pt_pool.tile([128, S], FP, tag="yT", bufs=2)
                        nc.vector.tensor_copy(yT[:], ptr[:, 0:S])
                        mu = pt_pool.tile([128, 1], FP, tag="mu", bufs=2)
                        nc.vector.tensor_reduce(mu[:], yT[:], mybir.AxisListType.X, ALU.add)
                        nc.vector.tensor_scalar_mul(mu[:], mu[:], 1.0 / S)
                        xc = pt_pool.tile([128, S], FP, tag="xc", bufs=2)
                        nc.vector.tensor_scalar_sub(xc[:], yT[:], mu[:])
                        scr2 = pt_pool.tile([128, S], FP, tag="scr2", bufs=2)
                        vv = pt_pool.tile([128, 1], FP, tag="vv", bufs=2)
                        nc.scalar.activation(scr2[:], xc[:], AF.Square, accum_out=vv[:])
                        nc.vector.tensor_scalar(vv[:], vv[:], 1.0 / S, 1e-5, ALU.mult, ALU.add)
                        nc.scalar.sqrt(vv[:], vv[:])
                        nc.vector.reciprocal(vv[:], vv[:])
                        nc.vector.tensor_scalar_mul(xc[:], xc[:], vv[:])
                        ptr2 = psA.tile([128, 128], FP, tag="ptr2", bufs=2)
                        nc.tensor.transpose(ptr2[0:S, :], xc[:], ident[:])
                        nc.vector.tensor_copy(yln[:, i * 128:(i + 1) * 128], ptr2[0:S, :])

                s2ib_sb = load_pcol("s2ib", INNER, 16)
                Dp_sb = load_pcol("Dp", INNER, 16)
                s2iw_sb = pm.tile([S, INNER], BF, name="s2iw_sb")
                nc.sync.dma_start(s2iw_sb[:], dp["s2iw"][:])

                # ---- s2i + gate sigmoid + pre_out assembly ----
                with nc.named_scope("premix"), tc.tile_pool(name="ps5", bufs=1, space="PSUM") as psA:
                    pre = []
                    for m in range(MB):
                        q, mi = divmod(m, 4)
                        if mi == 0:
                            wq = pt_pool.tile([128, KB, 512], BF, tag="wslab", bufs=2,
                                              name=f"wipg{q}")
                            for kb in range(KB):
                                nc.gpsimd.dma_start(
                                    wq[:, kb, :], dp["ipw"][kb * 128:(kb + 1) * 128,
                                                            2048 + q * 512:2048 + (q + 1) * 512])
                        ps = psA.tile([128, 512], FP, tag="ps", bufs=2)
                        nc.tensor.matmul(ps[:], s2iw_sb[:, m * 128:(m + 1) * 128], yln[:],
                                         start=True, stop=True)
                        pg = psA.tile([128, 512], FP, tag="pg", bufs=2)
                        for kb in range(KB):
                            nc.tensor.matmul(pg[:], wq[:, kb, mi * 128:(mi + 1) * 128],
                                             hT[kb][:, HALO:NH],
                                             start=(kb == 0), stop=(kb == KB - 1))
                        sg = pt_pool.tile([128, OWN], FP, tag="sg", bufs=2)
                        nc.scalar.activation(sg[:], pg[:], AF.Sigmoid,
                                             bias=ipb_sb[:, MB + m:MB + m + 1])
                        tmp = pt_pool.tile([128, OWN], FP, tag="tmp", bufs=2)
                        nc.vector.tensor_scalar(tmp[:], xm[m][:, HALO:NH],
                                                Dp_sb[:, m:m + 1], None, ALU.mult)
                        nc.vector.scalar_tensor_tensor(tmp[:], ps[:], s2ib_sb[:, m:m + 1],
                                                       tmp[:], ALU.add, ALU.add)
                        pre_m = pm.tile([128, OWN], BF, tag=f"xm{m}", name=f"pre{m}")
                        nc.vector.tensor_mul(pre_m[:], tmp[:], sg[:])
                        pre.append(pre_m)

                obb_sb = load_row("ob", D, BF)
                gw_sb = po.tile([128, KB, E], FP)  # [p, kb, e]
                nc.sync.dma_start(gw_sb[:], dp["gw"].ap().rearrange("(kb p) e -> p kb e", p=128))
                gb_sb = load_row("gb", E, FR)

                # ---- out projection (ow loaded ONCE, kb-outer) ----
                with nc.named_scope("outproj"), tc.tile_pool(name="ps6", bufs=1, space="PSUM") as psO:
                    pot = [[psO.tile([128, 512], FP, tag=f"po{t_}n{nb}", bufs=1,
                                     name=f"po{t_}n{nb}") for nb in range(2)]
                           for t_ in range(OTB)]
                    for kb in range(MB):
                        owt = pt_pool.tile([128, D], BF, tag="owt", bufs=3)
                        nc.sync.dma_start(owt[:], dp["ow"][kb * 128:(kb + 1) * 128, :])
                        for nb in range(2):
                            for tb in range(OTB):
                                nc.tensor.matmul(pot[tb][nb][:],
                                                 pre[kb][:, tb * 128:(tb + 1) * 128],
                                                 owt[:, nb * 512:(nb + 1) * 512],
                                                 start=(kb == 0), stop=False)
                    for tb in range(OTB):
                        for nb in range(2):
                            nc.tensor.matmul(pot[tb][nb][:], ones1b[:],
                                             obb_sb[:, nb * 512:(nb + 1) * 512],
                                             start=False, stop=True)
                            nc.vector.tensor_add(xmid[tb][:, nb * 512:(nb + 1) * 512],
                                                 pot[tb][nb][:],
                                                 xo[tb][:, nb * 512:(nb + 1) * 512])

                # ---- per-tb: rms2 + h2T + gather (AG issued ASAP), then gating ----
                h2T_all = [pt_pool.tile([128, 128], FP, tag=f"h2T{i}", bufs=1,
                                        name=f"h2T{i}") for i in range(OTB * KB)]
                with nc.named_scope("gating"), tc.tile_pool(name="ps7", bufs=1, space="PSUM") as psA:
                    for tb in range(OTB):
                        scr = pt_pool.tile([128, D], FP, tag="scr", bufs=2)
                        sq = pt_pool.tile([128, 1], FP, tag="sq", bufs=2)
                        nc.scalar.activation(scr[:], xmid[tb][:], AF.Square, accum_out=sq[:])
                        nr = pt_pool.tile([128, 1], FP, tag="nr", bufs=2)
                        nc.vector.tensor_scalar(nr[:], sq[:], 1.0 / D, 1e-6, ALU.mult, ALU.add)
                        nc.scalar.sqrt(nr[:], nr[:])
                        nc.vector.reciprocal(nr[:], nr[:])
                        h2 = pt_pool.tile([128, D], FP, tag="xt", bufs=2, name="h2")
                        nc.vector.tensor_scalar(h2[:], xmid[tb][:], nr[:], None, ALU.mult)
                        for kb in range(KB):
                            ptr = psA.tile([128, 128], FP, tag="ptr", bufs=2)
                            nc.tensor.transpose(ptr[:], h2[:, kb * 128:(kb + 1) * 128], ident[:])
                            h2T_t = h2T_all[tb * KB + kb]
                            nc.vector.tensor_copy(h2T_t[:], ptr[:])
                            nc.vector.tensor_copy(h2own[kb][:, tb * 128:(tb + 1) * 128],
                                                  h2T_t[:])
                            nc.sync.dma_start(
                                gth_in[tb][kb * 128:(kb + 1) * 128, :],
                                h2own[kb][:, tb * 128:(tb + 1) * 128])
                        nc.gpsimd.collective_compute(
                            "AllGather", ALU.bypass, replica_groups=rg,
                            ins=[gth_in[tb].opt()], outs=[gth_out[tb].opt()])
                    for tb in range(OTB):
                        pl = psA.tile([128, E], FP, tag="pl", bufs=2)
                        for kb in range(KB):
                            nc.tensor.matmul(pl[:], h2T_all[tb * KB + kb][:], gw_sb[:, kb, :],
                                             start=(kb == 0), stop=False)
                        nc.tensor.matmul(pl[:], ones1[:], gb_sb[:], start=False, stop=True)
                        # top-2-of-4 gating
                        m1 = pt_pool.tile([128, 1], FP, tag="m1", bufs=2)
                        nc.vector.tensor_reduce(m1[:], pl[:], mybir.AxisListType.X, ALU.max)
                        eq1 = pt_pool.tile([128, E], FP, tag="eq1", bufs=2)
                        nc.vector.tensor_scalar(eq1[:], pl[:], m1[:], None, ALU.is_equal)
                        msk = pt_pool.tile([128, E], FP, tag="msk", bufs=2)
                        nc.vector.scalar_tensor_tensor(msk[:], eq1[:], -1e30, pl[:],
                                                       ALU.mult, ALU.add)
                        m2 = pt_pool.tile([128, 1], FP, tag="m2", bufs=2)
                        nc.vector.tensor_reduce(m2[:], msk[:], mybir.AxisListType.X, ALU.max)
                        eq2 = pt_pool.tile([128, E], FP, tag="eq2", bufs=2)
                        nc.vector.tensor_scalar(eq2[:], msk[:], m2[:], None, ALU.is_equal)
                        dd = pt_pool.tile([128, 1], FP, tag="dd", bufs=2)
                        nc.vector.tensor_sub(dd[:], m2[:], m1[:])
                        p2 = pt_pool.tile([128, 1], FP, tag="p2", bufs=2)
                        nc.scalar.activation(p2[:], dd[:], AF.Sigmoid)
                        p1b = pt_pool.tile([128, 1], FP, tag="p1b", bufs=2)
                        nc.scalar.activation(p1b[:], p2[:], AF.Identity, bias=1.0, scale=-1.0)
                        nc.vector.tensor_scalar(wv_sb[tb][:], eq1[:], p1b[:], None, ALU.mult)
                        nc.vector.scalar_tensor_tensor(wv_sb[tb][:], eq2[:], p2[:], wv_sb[tb][:],
                                                       ALU.mult, ALU.add)
                        nc.sync.dma_start(gtw_in[tb * 128:(tb + 1) * 128, :], wv_sb[tb][:])
                    with nc.named_scope("gather"):
                        nc.gpsimd.collective_compute(
                            "AllGather", ALU.bypass, replica_groups=rg,
                            ins=[gtw_in.opt()], outs=[gtw_out.opt()])

            # =======================================================
            # MoE (full expert per core, token-half group of 4)
            # =======================================================
            with (
                tc.tile_pool(name="moe", bufs=1) as pq,
                tc.tile_pool(name="psC", bufs=1, space="PSUM") as psC,
            ):
                esel = po.tile([128, E], FP)
                nc.sync.dma_start(esel[:], dp["esel"][:])
                rmask = po.tile([128, 4], FP)
                nc.sync.dma_start(rmask[:], dp["rmask"][:])
                eb1_sb = load_pcol("eb1", 2 * HH, 32)
                eb2h_sb = load_row("eb2h", D, BF)
                HB = 2 * HH // 128  # 32 hid blocks
                with nc.named_scope("moe_w"):
                    ew1_sb = [pq.tile([128, 2 * HH], BF, name=f"ew1_{kb}", tag=f"ew1_{kb}")
                              for kb in range(KB)]
                    for kb in range(KB):
                        nc.gpsimd.dma_start(ew1_sb[kb][:], dp["ew1"][kb * 128:(kb + 1) * 128, :])

                with nc.named_scope("moe"):
                    for r in range(4):
                        # h2 for this round: own quarter lives in SBUF already
                        h2r = []
                        for kb in range(KB):
                            t = pq.tile([128, OWN], BF, tag=f"h2r{kb}", bufs=2)
                            for t_ in range(OTB):
                                nc.sync.dma_start(
                                    t[:, t_ * 128:(t_ + 1) * 128],
                                    gth_out[t_][r * D + kb * 128: r * D + (kb + 1) * 128, :])
                            h2r.append(t)
                        hid = []
                        for h in range(HB):
                            ph = psC.tile([128, 512], FP, tag="ph", bufs=2)
                            for kb in range(KB):
                                nc.tensor.matmul(ph[:], ew1_sb[kb][:, h * 128:(h + 1) * 128],
                                                 h2r[kb][:], start=(kb == 0), stop=(kb == KB - 1))
                            ht = pq.tile([128, OWN], BF, tag=f"hid{h}", bufs=1)
                            nc.scalar.activation(ht[:], ph[:], AF.Gelu, bias=eb1_sb[:, h:h + 1])
                            hid.append(ht)
                        # per-token weight for this core's expert
                        wvr = pq.tile([128, OTB, E], FP, tag="wvr", bufs=2)
                        nc.sync.dma_start(
                            wvr[:], gtw_out[r * OWN:(r + 1) * OWN, :]
                            .rearrange("(tb p) e -> p tb e", p=128))
                        ws = []
                        for tb in range(OTB):
                            wm_t = pq.tile([128, E], FP, tag="wm", bufs=2)
                            nc.vector.tensor_mul(wm_t[:], wvr[:, tb, :], esel[:])
                            ws_t = pq.tile([128, 1], FP, tag=f"ws{tb}", bufs=2)
                            nc.vector.tensor_reduce(ws_t[:], wm_t[:], mybir.AxisListType.X,
                                                    ALU.add)
                            ws.append(ws_t)
                        for nb in range(2):
                            peo = [psC.tile([128, 512], FP, tag=f"peo{t_}", bufs=1,
                                            name=f"peo{t_}") for t_ in range(OTB)]
                            for hc in range(4):
                                ew2c = pq.tile([128, 8, 512], BF, tag="ew2c", bufs=2)
                                for hh in range(8):
                                    nc.gpsimd.dma_start(
                                        ew2c[:, hh, :],
                                        dp["ew2"][(hc * 8 + hh) * 128:(hc * 8 + hh + 1) * 128,
                                                  nb * 512:(nb + 1) * 512])
                                for hh in range(8):
                                    h = hc * 8 + hh
                                    for tb in range(OTB):
                                        nc.tensor.matmul(
                                            peo[tb][:], hid[h][:, tb * 128:(tb + 1) * 128],
                                            ew2c[:, hh, :], start=(h == 0), stop=False)
                            for tb in range(OTB):
                                nc.tensor.matmul(peo[tb][:], ones1b[:],
                                                 eb2h_sb[:, nb * 512:(nb + 1) * 512],
                                                 start=False, stop=True)
                                wout = pq.tile([128, 512], FP, tag="wout", bufs=3)
                                nc.vector.tensor_scalar(wout[:], peo[tb][:], ws[tb][:],
                                                        None, ALU.mult)
                                nc.vector.scalar_tensor_tensor(
                                    wout[:], xmid[tb][:, nb * 512:(nb + 1) * 512],
                                    rmask[:, r:r + 1], wout[:], ALU.mult, ALU.add)
                                nc.sync.dma_start(
                                    rs_in[r][tb * 128:(tb + 1) * 128,
                                             nb * 512:(nb + 1) * 512], wout[:])
                        nc.gpsimd.collective_compute(
                            "ReduceScatter", ALU.add, replica_groups=rg,
                            ins=[rs_in[r].opt()], outs=[rs_out[r].opt()])
                        nc.sync.dma_start(out_d[r * 128:(r + 1) * 128, :], rs_out[r][:])

    nc.compile()
    return nc


def host_prep(inputs):
    """Build the 8 per-core input maps from full inputs."""
    import ml_dtypes
    f32 = np.float32
    bf16 = ml_dtypes.bfloat16
    x = np.ascontiguousarray(np.asarray(inputs["x"], f32).reshape(B * T, D))
    n1 = np.asarray(inputs["norm1_w"], f32)
    n2 = np.asarray(inputs["norm2_w"], f32)
    ipw = np.ascontiguousarray(
        (np.asarray(inputs["in_proj_w"], f32) * n1[:, None]).astype(bf16))
    gw = np.ascontiguousarray(np.asarray(inputs["gate_w"], f32) * n2[:, None])
    ew1f = np.asarray(inputs["e_w1"], f32) * n2[None, :, None]
    ew1b = ew1f.astype(bf16)
    ew2b = np.asarray(inputs["e_w2"], f32).astype(bf16)
    dbw = np.ascontiguousarray(np.concatenate(
        [np.asarray(inputs["dt_w"], f32), np.asarray(inputs["bp_w"], f32)], axis=1))
    ident = np.eye(128, dtype=f32)
    ones1 = np.ones((1, 128), f32)
    shared = {
        "ipw": ipw, "ipb": np.asarray(inputs["in_proj_b"], f32),
        "cw": np.ascontiguousarray(np.asarray(inputs["conv_w"], f32)[:, 0, :]),
        "cb": np.asarray(inputs["conv_b"], f32),
        "dbw": dbw,
        "dtb": np.asarray(inputs["dt_b"], f32), "bpb": np.asarray(inputs["bp_b"], f32),
        "cpw": np.asarray(inputs["cp_w"], f32), "cpb": np.asarray(inputs["cp_b"], f32),
        "s2iw": np.asarray(inputs["s2i_w"], f32).astype(bf16),
        "s2ib": np.asarray(inputs["s2i_b"], f32),
        "Dp": np.asarray(inputs["D_param"], f32),
        "ow": np.asarray(inputs["out_w"], f32).astype(bf16),
        "ob": np.asarray(inputs["out_b"], f32).astype(bf16),
        "gw": gw, "gb": np.asarray(inputs["gate_b"], f32),
        "ident": ident, "ones1": ones1, "ones1b": ones1.astype(bf16),
    }
    eb1 = np.asarray(inputs["e_b1"], f32)
    eb2 = np.asarray(inputs["e_b2"], f32)
    in_maps = []
    for c in range(N_CORES):
        e, th = c // 2, c % 2
        g0 = th * (B * T // 2) + e * OWN
        if e == 0:
            x_sh = np.concatenate([np.zeros((HALO, D), f32), x[g0:g0 + OWN]])
        else:
            x_sh = x[g0 - HALO:g0 + OWN]
        m = dict(shared)
        m["x_sh"] = np.ascontiguousarray(x_sh)
        m["ew1"] = np.ascontiguousarray(ew1b[e])
        m["eb1"] = np.ascontiguousarray(eb1[e])
        m["ew2"] = np.ascontiguousarray(ew2b[e])
        m["eb2h"] = np.ascontiguousarray(eb2[e].astype(bf16))
        esel = np.zeros((128, E), f32)
        esel[:, e] = 1.0
        m["esel"] = esel
        rmask = np.zeros((128, 4), f32)
        rmask[:, e] = 1.0
        m["rmask"] = rmask
        in_maps.append(m)
    return in_maps


def unshard_out(results):
    """results: list of 8 dicts with 'out' [OWN, D]; rows r*128+i of core c
    hold global token (c%2)*2048 + r*512 + (c//2)*128 + i."""
    full = np.empty((B * T, D), np.float32)
    for c in range(N_CORES):
        e, th = c // 2, c % 2
        oc = results[c]["out"]
        for r in range(4):
            full[th * 2048 + r * OWN + e * 128: th * 2048 + r * OWN + (e + 1) * 128] = \
                oc[r * 128:(r + 1) * 128]
    return full.reshape(B, T, D)


_NC_CACHE = {}


def _get_nc():
    if "nc" not in _NC_CACHE:
        _NC_CACHE["nc"] = build(debug_outputs=False)
    return _NC_CACHE["nc"]


def kernel(**inputs) -> np.ndarray:
    """Full-input entry point: shards across 8 NeuronCores, runs the Bass
    kernel SPMD, reassembles the full [2, 2048, 1024] output."""
    import sys, types
    try:  # NTFF profile hook shim (missing antenv.axon_hooks in this image)
        import antenv.axon_hooks  # noqa: F401
    except ImportError:
        try:
            import antenv
            from trn_agent_boot.trn_boot import _ntff_profile_via_ctypes
            mod = types.ModuleType("antenv.axon_hooks")
            try:
                _hook = _ntff_profile_via_ctypes("/opt/axon/libaxon_pjrt.so")
            except Exception:
                _hook = None
            mod.get_axon_ntff_profile_hook = lambda: _hook
            mod.set_axon_ntff_profile_hook = lambda h: None
            sys.modules["antenv.axon_hooks"] = mod
            antenv.axon_hooks = mod
        except Exception:
            pass
    from concourse.bass_utils import run_bass_kernel_spmd

    nc = _get_nc()
    in_maps = host_prep(inputs)
    res = run_bass_kernel_spmd(nc, in_maps, core_ids=list(range(N_CORES)))
    out = unshard_out(res.results)
    return out.astype(np.float32)


# revision 13
# speedup vs baseline: 1.3020x; 1.3020x over previous
"""Bass kernel builder for nn_MixtureOfMambaBlock — 8-core SPMD.

Sharding: tokens 8-way (512/core + 128 halo for conv+scan warmup); mixer fully
local per core (weights replicated, bf16 matmuls; gate-logit path kept f32).
Post-mixer h2 all-gathered (bf16), MoE expert-parallel (one expert per core,
dense over the 2048-token half), weighted partials reduce-scattered back.
"""
import numpy as np
import concourse.bass as bass
import concourse.bacc as bacc
import concourse.mybir as mybir
import concourse.tile as tile

FP = mybir.dt.float32
FR = mybir.dt.float32r
BF = mybir.dt.bfloat16
F8 = mybir.dt.float8e4
DR = mybir.MatmulPerfMode.DoubleRow
AF = mybir.ActivationFunctionType
ALU = mybir.AluOpType

B, T, D = 2, 2048, 1024
S, INNER = 64, 2048
E, HH = 4, 2048          # experts, hid-half width
OWN, HALO = 512, 128
NH = OWN + HALO          # 640
KB = D // 128            # 8  d-blocks
MB = INNER // 128        # 16 inner-blocks
OTB = OWN // 128         # 4  own-token blocks
N_CORES = 8

INPUT_SPECS = {
    "x_sh": ([NH, D], FP),
    "ipw": ([D, 2 * INNER], BF), "ipb": ([2 * INNER], FP),
    "cw": ([INNER, 3], FP), "cb": ([INNER], FP),
    "dbw": ([INNER, 128], FR),  # dt_w || bp_w stacked on output dim
    "dtb": ([S], FP), "bpb": ([S], FP),
    "cpw": ([INNER, S], FR), "cpb": ([S], FP),
    "s2iw": ([S, INNER], BF), "s2ib": ([INNER], FP),
    "Dp": ([INNER], FP),
    "ow": ([INNER, D], BF), "ob": ([D], BF),
    "gw": ([D, E], FP), "gb": ([E], FR),
    "ew1": ([D, 2 * HH], BF), "eb1": ([2 * HH], FP),
    "ew2": ([2 * HH, D], F8), "eb2h": ([D], BF),
    "esel": ([128, E], FP),
    "rmask": ([128, 4], FP),
    "ident": ([128, 128], FP),
    "ones1": ([1, 128], FR),
    "ones1b": ([1, 128], BF),
}


def build(debug_outputs=False):
    nc = bacc.Bacc("TRN2", target_bir_lowering=False, debug=False,
                   num_devices=N_CORES)
    dp = {}
    for name, (shape, dt) in INPUT_SPECS.items():
        dp[name] = nc.dram_tensor(name, shape, dt, kind="ExternalInput")
    out_d = nc.dram_tensor("out", [OWN, D], FP, kind="ExternalOutput")

    rg = [[0, 2, 4, 6], [1, 3, 5, 7]]

    with tile.TileContext(nc) as tc:
        with (
            tc.tile_pool(name="outer", bufs=1) as po,
            tc.tile_pool(name="dram", bufs=1, space="DRAM") as pdram,
        ):
            # ---------- DRAM bounce buffers for collectives ----------
            gth_in = [pdram.tile([D, 128], BF, name=f"gth_in{t_}") for t_ in range(OTB)]
            gth_out = [pdram.tile([4 * D, 128], BF, name=f"gth_out{t_}")
                       for t_ in range(OTB)]
            gtw_in = pdram.tile([OWN, E], FP)
            gtw_out = pdram.tile([4 * OWN, E], FP)
            rs_in = [pdram.tile([OWN, D], FP, name=f"rs_in{r}") for r in range(4)]
            rs_out = [pdram.tile([128, D], FP, name=f"rs_out{r}") for r in range(4)]

            # ---------- constants / small weights ----------
            ident = po.tile([128, 128], FP)
            nc.sync.dma_start(ident[:], dp["ident"][:])

            def load_pcol(name, n, blocks):  # [n*128] -> [128, blocks] (col b = block b)
                t = po.tile([128, blocks], FP, name=f"{name}_sb")
                nc.sync.dma_start(
                    t[:], dp[name].ap().rearrange("(m p) -> p m", p=128))
                return t

            def load_vec1(name, n):  # [n] -> [n, 1]
                t = po.tile([n, 1], FP, name=f"{name}_sb")
                nc.sync.dma_start(t[:], dp[name].ap().rearrange("(s o) -> s o", o=1))
                return t

            def load_row(name, n, dt_=FP):  # [n] -> [1, n]
                t = po.tile([1, n], dt_, name=f"{name}_sb")
                nc.sync.dma_start(t[:], dp[name].ap().rearrange("(o s) -> o s", o=1))
                return t

            ones1 = po.tile([1, 128], FR)
            nc.sync.dma_start(ones1[:], dp["ones1"][:])
            ones1b = po.tile([1, 128], BF)
            nc.sync.dma_start(ones1b[:], dp["ones1b"][:])

            # persistent activations (live into MoE phase)
            xo = [po.tile([128, D], FP, name=f"xo{t_}", tag=f"xo{t_}") for t_ in range(OTB)]
            xmid = [po.tile([128, D], FP, name=f"xmid{t_}", tag=f"xmid{t_}") for t_ in range(OTB)]
            h2own = [po.tile([128, OWN], BF, name=f"h2own{kb}", tag=f"h2own{kb}")
                     for kb in range(KB)]
            wv_sb = [po.tile([128, E], FP, name=f"wv{t_}", tag=f"wv{t_}") for t_ in range(OTB)]

            # =======================================================
            # MIXER
            # =======================================================
            with (
                tc.tile_pool(name="mixer", bufs=1) as pm,
                tc.tile_pool(name="mixt", bufs=1) as pt_pool,
            ):
                hT = [pm.tile([128, NH], BF, name=f"hT{kb}", tag=f"hT{kb}") for kb in range(KB)]
                xm = [pm.tile([128, NH], FR, name=f"xm{m}", tag=f"xm{m}") for m in range(MB)]

                # ---- rmsnorm1 + transpose to hT (bf16) ----
                with nc.named_scope("rms1"), tc.tile_pool(name="ps1", bufs=1, space="PSUM") as psA:
                    for tb in range(NH // 128):
                        if tb == 0:
                            xt = pt_pool.tile([128, D], FP, tag="xt", bufs=2)
                        else:
                            xt = xo[tb - 1]
                        nc.sync.dma_start(xt[:], dp["x_sh"][tb * 128:(tb + 1) * 128, :])
                        scr = pt_pool.tile([128, D], FP, tag="scr", bufs=2)
                        sq = pt_pool.tile([128, 1], FP, tag="sq", bufs=2)
                        nc.scalar.activation(scr[:], xt[:], AF.Square, accum_out=sq[:])
                        nr = pt_pool.tile([128, 1], FP, tag="nr", bufs=2)
                        nc.vector.tensor_scalar(nr[:], sq[:], 1.0 / D, 1e-6, ALU.mult, ALU.add)
                        nc.scalar.sqrt(nr[:], nr[:])
                        nc.vector.reciprocal(nr[:], nr[:])
                        h_t = pt_pool.tile([128, D], FP, tag="scr", bufs=2)
                        nc.vector.tensor_scalar(h_t[:], xt[:], nr[:], None, ALU.mult)
                        for kb in range(KB):
                            ptr = psA.tile([128, 128], FP, tag="ptr", bufs=2)
                            nc.tensor.transpose(ptr[:], h_t[:, kb * 128:(kb + 1) * 128], ident[:])
                            nc.vector.tensor_copy(hT[kb][:, tb * 128:(tb + 1) * 128], ptr[:])

                ipb_sb = load_pcol("ipb", 2 * INNER, 32)
                cb_sb = load_pcol("cb", INNER, 16)
                cw_sb = po.tile([128, 16, 3], FP)  # [p, m, k]
                nc.sync.dma_start(cw_sb[:], dp["cw"].ap().rearrange("(m p) k -> p m k", p=128))

                # ---- in_proj (x_main half) + conv + silu ----
                with nc.named_scope("in_proj"), tc.tile_pool(name="ps2", bufs=1, space="PSUM") as psA:
                    for q in range(4):
                        wq = pt_pool.tile([128, KB, 512], BF, tag="wslab", bufs=2,
                                          name=f"wip{q}")
                        for kb in range(KB):
                            nc.gpsimd.dma_start(
                                wq[:, kb, :], dp["ipw"][kb * 128:(kb + 1) * 128,
                                                        q * 512:(q + 1) * 512])
                        for mi in range(4):
                            m = q * 4 + mi
                            xzp = pt_pool.tile([128, NH + 2], FP, tag="xzp", bufs=2)
                            nc.vector.memset(xzp[:, 0:2], 0.0)
                            for n0, nw in ((0, 512), (512, 128)):
                                px = psA.tile([128, 512], FP, tag="px", bufs=2)
                                for kb in range(KB):
                                    nc.tensor.matmul(px[:, 0:nw],
                                                     wq[:, kb, mi * 128:(mi + 1) * 128],
                                                     hT[kb][:, n0:n0 + nw],
                                                     start=(kb == 0), stop=(kb == KB - 1))
                                nc.scalar.activation(xzp[:, 2 + n0:2 + n0 + nw], px[:, 0:nw],
                                                     AF.Identity, bias=ipb_sb[:, m:m + 1])
                            cv = pt_pool.tile([128, NH], FP, tag="cv", bufs=2)
                            nc.vector.tensor_scalar(cv[:], xzp[:, 0:NH], cw_sb[:, m, 0:1],
                                                    None, ALU.mult)
                            nc.vector.scalar_tensor_tensor(cv[:], xzp[:, 1:1 + NH],
                                                           cw_sb[:, m, 1:2], cv[:],
                                                           ALU.mult, ALU.add)
                            nc.vector.scalar_tensor_tensor(cv[:], xzp[:, 2:2 + NH],
                                                           cw_sb[:, m, 2:3], cv[:],
                                                           ALU.mult, ALU.add)
                            sgc = pt_pool.tile([128, NH], FP, tag="sgc", bufs=2)
                            nc.scalar.activation(sgc[:], cv[:], AF.Sigmoid, bias=cb_sb[:, m:m + 1])
                            nc.vector.scalar_tensor_tensor(xm[m][:], cv[:], cb_sb[:, m:m + 1],
                                                           sgc[:], ALU.add, ALU.mult)

                dtb_sb = load_vec1("dtb", S)
                bpb_sb = load_vec1("bpb", S)
                cpb_sb = load_vec1("cpb", S)
                dbw_sb = pm.tile([128, MB, 128], FR, name="dbw_sb")
                nc.sync.dma_start(dbw_sb[:], dp["dbw"].ap().rearrange("(kb p) s -> p kb s", p=128))
                cpw_sb = pm.tile([128, MB, S], FR, name="cpw_sb")
                nc.sync.dma_start(cpw_sb[:], dp["cpw"].ap().rearrange("(kb p) s -> p kb s", p=128))

                # ---- dt/B/C projections + scan ----
                with nc.named_scope("scan"), tc.tile_pool(name="ps3", bufs=1, space="PSUM") as psA:
                    dt_t = pt_pool.tile([S, NH], FP, tag="dt")
                    a_t = pt_pool.tile([S, NH], FP, tag="a")
                    b_t = pt_pool.tile([S, NH], FP, tag="b")
                    c_t = pt_pool.tile([S, NH], FP, tag="c")
                    for n0, nw in ((0, 320), (320, 320)):
                        pzdb = psA.tile([128, 320], FP, tag="pzdb", bufs=2)
                        for kb in range(MB):
                            nc.tensor.matmul(pzdb[:, 0:nw], dbw_sb[:, kb, :],
                                             xm[kb][:, n0:n0 + nw],
                                             start=(kb == 0), stop=(kb == MB - 1))
                        nc.scalar.activation(dt_t[:, n0:n0 + nw], pzdb[0:S, 0:nw],
                                             AF.Sigmoid, bias=dtb_sb[:])
                        nc.vector.scalar_tensor_tensor(b_t[:, n0:n0 + nw], pzdb[S:128, 0:nw],
                                                       bpb_sb[:], dt_t[:, n0:n0 + nw],
                                                       ALU.add, ALU.mult)
                        pzc = psA.tile([S, 320], FP, tag="pzc", bufs=2)
                        for kb in range(MB):
                            nc.tensor.matmul(pzc[:, 0:nw], cpw_sb[:, kb, :],
                                             xm[kb][:, n0:n0 + nw],
                                             start=(kb == 0), stop=(kb == MB - 1))
                        nc.scalar.activation(c_t[:, n0:n0 + nw], pzc[:, 0:nw], AF.Identity,
                                             bias=cpb_sb[:])
                    nc.scalar.activation(a_t[:], dt_t[:], AF.Identity, bias=1.0, scale=-1.0)
                    st_t = pt_pool.tile([S, NH], FP, tag="st")
                    nc.vector.tensor_tensor_scan(st_t[:], a_t[:], b_t[:], 0.0,
                                                 ALU.mult, ALU.add)
                    y_t = pt_pool.tile([S, OWN], FP, tag="dt", name="y_t")
                    nc.vector.tensor_mul(y_t[:], c_t[:, HALO:NH], st_t[:, HALO:NH])

                # ---- layernorm over S (transpose - LN - transpose back) ----
                with nc.named_scope("ln"), tc.tile_pool(name="ps4", bufs=1, space="PSUM") as psA:
                    yln = pt_pool.tile([S, OWN], BF, tag="a", name="yln")
                    for i in range(OTB):
                        ptr = psA.tile([128, 128], FP, tag="ptr", bufs=2)
                        nc.tensor.transpose(ptr[:, 0:S], y_t[:, i * 128:(i + 1) * 128],
                                            ident[0:S, 0:S])
                        yT = pt_pool.tile([128, S], FP, tag="yT", bufs=2)
                        nc.vector.tensor_copy(yT[:], ptr[:, 0:S])
                        mu = pt_pool.tile([128, 1], FP, tag="mu", bufs=2)
                        nc.vector.tensor_reduce(mu[:], yT[:], mybir.AxisListType.X, ALU.add)
                        nc.vector.tensor_scalar_mul(mu[:], mu[:], 1.0 / S)
                        xc = pt_pool.tile([128, S], FP, tag="xc", bufs=2)
                        nc.vector.tensor_scalar_sub(xc[:], yT[:], mu[:])
                        scr2 = pt_pool.tile([128, S], FP, tag="scr2", bufs=2)
                        vv = pt_pool.tile([128, 1], FP, tag="vv", bufs=2)
                        nc.scalar.activation(scr2[:], xc[:], AF.Square, accum_out=vv[:])
                        nc.vector.tensor_scalar(vv[:], vv[:], 1.0 / S, 1e-5, ALU.mult, ALU.add)
                        nc.scalar.sqrt(vv[:], vv[:])
                        nc.vector.reciprocal(vv[:], vv[:])
                        nc.vector.tensor_scalar_mul(xc[:], xc[:], vv[:])
                        ptr2 = psA.tile([128, 128], FP, tag="ptr2", bufs=2)
                        nc.tensor.transpose(ptr2[0:S, :], xc[:], ident[:])
                        nc.vector.tensor_copy(yln[:, i * 128:(i + 1) * 128], ptr2[0:S, :])

                s2ib_sb = load_pcol("s2ib", INNER, 16)
                Dp_sb = load_pcol("Dp", INNER, 16)
                s2iw_sb = pm.tile([S, INNER], BF, name="s2iw_sb")
                nc.sync.dma_start(s2iw_sb[:], dp["s2iw"][:])

                # ---- s2i + gate sigmoid + pre_out assembly ----
                with nc.named_scope("premix"), tc.tile_pool(name="ps5", bufs=1, space="PSUM") as psA:
                    pre = []
                    for m in range(MB):
                        q, mi = divmod(m, 4)
                        if mi == 0:
                            wq = pt_pool.tile([128, KB, 512], BF, tag="wslab", bufs=2,
                                              name=f"wipg{q}")
                            for kb in range(KB):
                                nc.gpsimd.dma_start(
                                    wq[:, kb, :], dp["ipw"][kb * 128:(kb + 1) * 128,
                                                            2048 + q * 512:2048 + (q + 1) * 512])
                        ps = psA.tile([128, 512], FP, tag="ps", bufs=2)
                        nc.tensor.matmul(ps[:], s2iw_sb[:, m * 128:(m + 1) * 128], yln[:],
                                         start=True, stop=True)
                        pg = psA.tile([128, 512], FP, tag="pg", bufs=2)
                        for kb in range(KB):
                            nc.tensor.matmul(pg[:], wq[:, kb, mi * 128:(mi + 1) * 128],
                                             hT[kb][:, HALO:NH],
                                             start=(kb == 0), stop=(kb == KB - 1))
                        sg = pt_pool.tile([128, OWN], FP, tag="sg", bufs=2)
                        nc.scalar.activation(sg[:], pg[:], AF.Sigmoid,
                                             bias=ipb_sb[:, MB + m:MB + m + 1])
                        tmp = pt_pool.tile([128, OWN], FP, tag="tmp", bufs=2)
                        nc.vector.tensor_scalar(tmp[:], xm[m][:, HALO:NH],
                                                Dp_sb[:, m:m + 1], None, ALU.mult)
                        nc.vector.scalar_tensor_tensor(tmp[:], ps[:], s2ib_sb[:, m:m + 1],
                                                       tmp[:], ALU.add, ALU.add)
                        pre_m = pm.tile([128, OWN], BF, tag=f"xm{m}", name=f"pre{m}")
                        nc.vector.tensor_mul(pre_m[:], tmp[:], sg[:])
                        pre.append(pre_m)

                obb_sb = load_row("ob", D, BF)
                gw_sb = po.tile([128, KB, E], FP)  # [p, kb, e]
                nc.sync.dma_start(gw_sb[:], dp["gw"].ap().rearrange("(kb p) e -> p kb e", p=128))
                gb_sb = load_row("gb", E, FR)

                # ---- out projection (ow loaded ONCE, kb-outer) ----
                with nc.named_scope("outproj"), tc.tile_pool(name="ps6", bufs=1, space="PSUM") as psO:
                    pot = [[psO.tile([128, 512], FP, tag=f"po{t_}n{nb}", bufs=1,
                                     name=f"po{t_}n{nb}") for nb in range(2)]
                           for t_ in range(OTB)]
                    for kb in range(MB):
                        owt = pt_pool.tile([128, D], BF, tag="owt", bufs=3)
                        nc.sync.dma_start(owt[:], dp["ow"][kb * 128:(kb + 1) * 128, :])
                        for nb in range(2):
                            for tb in range(OTB):
                                nc.tensor.matmul(pot[tb][nb][:],
                                                 pre[kb][:, tb * 128:(tb + 1) * 128],
                                                 owt[:, nb * 512:(nb + 1) * 512],
                                                 start=(kb == 0), stop=False)
                    for tb in range(OTB):
                        for nb in range(2):
                            nc.tensor.matmul(pot[tb][nb][:], ones1b[:],
                                             obb_sb[:, nb * 512:(nb + 1) * 512],
                                             start=False, stop=True)
                            nc.vector.tensor_add(xmid[tb][:, nb * 512:(nb + 1) * 512],
                                                 pot[tb][nb][:],
                                                 xo[tb][:, nb * 512:(nb + 1) * 512])

                # ---- per-tb: rms2 + h2T + gather (AG issued ASAP), then gating ----
                h2T_all = [pt_pool.tile([128, 128], FP, tag=f"h2T{i}", bufs=1,
                                        name=f"h2T{i}") for i in range(OTB * KB)]
                with nc.named_scope("gating"), tc.tile_pool(name="ps7", bufs=1, space="PSUM") as psA:
                    for tb in range(OTB):
                        scr = pt_pool.tile([128, D], FP, tag="scr", bufs=2)
                        sq = pt_pool.tile([128, 1], FP, tag="sq", bufs=2)
                        nc.scalar.activation(scr[:], xmid[tb][:], AF.Square, accum_out=sq[:])
                        nr = pt_pool.tile([128, 1], FP, tag="nr", bufs=2)
                        nc.vector.tensor_scalar(nr[:], sq[:], 1.0 / D, 1e-6, ALU.mult, ALU.add)
                        nc.scalar.sqrt(nr[:], nr[:])
                        nc.vector.reciprocal(nr[:], nr[:])
                        h2 = pt_pool.tile([128, D], FP, tag="xt", bufs=2, name="h2")
                        nc.vector.tensor_scalar(h2[:], xmid[tb][:], nr[:], None, ALU.mult)
                        for kb in range(KB):
                            ptr = psA.tile([128, 128], FP, tag="ptr", bufs=2)
                            nc.tensor.transpose(ptr[:], h2[:, kb * 128:(kb + 1) * 128], ident[:])
                            h2T_t = h2T_all[tb * KB + kb]
                            nc.vector.tensor_copy(h2T_t[:], ptr[:])
                            nc.vector.tensor_copy(h2own[kb][:, tb * 128:(tb + 1) * 128],
                                                  h2T_t[:])
                            nc.sync.dma_start(
                                gth_in[tb][kb * 128:(kb + 1) * 128, :],
                                h2own[kb][:, tb * 128:(tb + 1) * 128])
                        nc.gpsimd.collective_compute(
                            "AllGather", ALU.bypass, replica_groups=rg,
                            ins=[gth_in[tb].opt()], outs=[gth_out[tb].opt()])
                    for tb in range(OTB):
                        pl = psA.tile([128, E], FP, tag="pl", bufs=2)
                        for kb in range(KB):
                            nc.tensor.matmul(pl[:], h2T_all[tb * KB + kb][:], gw_sb[:, kb, :],
                                             start=(kb == 0), stop=False)
                        nc.tensor.matmul(pl[:], ones1[:], gb_sb[:], start=False, stop=True)
                        # top-2-of-4 gating
                        m1 = pt_pool.tile([128, 1], FP, tag="m1", bufs=2)
                        nc.vector.tensor_reduce(m1[:], pl[:], mybir.AxisListType.X, ALU.max)
                        eq1 = pt_pool.tile([128, E], FP, tag="eq1", bufs=2)
                        nc.vector.tensor_scalar(eq1[:], pl[:], m1[:], None, ALU.is_equal)
                        msk = pt_pool.tile([128, E], FP, tag="msk", bufs=2)
                        nc.vector.scalar_tensor_tensor(msk[:], eq1[:], -1e30, pl[:],
                                                       ALU.mult, ALU.add)
                        m2 = pt_pool.tile([128, 1], FP, tag="m2", bufs=2)
                        nc.vector.tensor_reduce(m2[:], msk[:], mybir.AxisListType.X, ALU.max)
                        eq2 = pt_pool.tile([128, E], FP, tag="eq2", bufs=2)
                        nc.vector.tensor_scalar(eq2[:], msk[:], m2[:], None, ALU.is_equal)
                        dd = pt_pool.tile([128, 1], FP, tag="dd", bufs=2)
                        nc.vector.tensor_sub(dd[:], m2[:], m1[:])
                        p2 = pt_pool.tile([128, 1], FP, tag="p2", bufs=2)
                        nc.scalar.activation(p2[:], dd[:], AF.Sigmoid)
                        p1b = pt_pool.tile([128, 1], FP, tag="p1b", bufs=2)
                        nc.scalar.activation(p1b[:], p2[:], AF.Identity, bias=1.0, scale=-1.0)
                        nc.vector.tensor_scalar(wv_sb[tb][:], eq1[:], p1b[:], None, ALU.mult)
                        nc.vector.scalar_tensor_tensor(wv_sb[tb][:], eq2[:], p2[:], wv_sb[tb][:],
                                                       ALU.mult, ALU.add)
                        nc.sync.dma_start(gtw_in[tb * 128:(tb + 1) * 128, :], wv_sb[tb][:])
                    with nc.named_scope("gather"):
                        nc.gpsimd.collective_compute(
                            "AllGather", ALU.bypass, replica_groups=rg,
                            ins=[gtw_in.opt()], outs=[gtw_out.opt()])

            # =======================================================
            # MoE (full expert per core, token-half group of 4)
            # =======================================================
            with (
                tc.tile_pool(name="moe", bufs=1) as pq,
                tc.tile_pool(name="psC", bufs=1, space="PSUM") as psC,
            ):
                esel = po.tile([128, E], FP)
                nc.sync.dma_start(esel[:], dp["esel"][:])
                rmask = po.tile([128, 4], FP)
                nc.sync.dma_start(rmask[:], dp["rmask"][:])
                eb1_sb = load_pcol("eb1", 2 * HH, 32)
                eb2h_sb = load_row("eb2h", D, BF)
                HB = 2 * HH // 128  # 32 hid blocks
                with nc.named_scope("moe_w"):
                    ew1_sb = [pq.tile([128, 2 * HH], BF, name=f"ew1_{kb}", tag=f"ew1_{kb}")
                              for kb in range(KB)]
                    for kb in range(KB):
                        nc.gpsimd.dma_start(ew1_sb[kb][:], dp["ew1"][kb * 128:(kb + 1) * 128, :])

                with nc.named_scope("moe"):
                    for r in range(4):
                        # h2 for this round: own quarter lives in SBUF already
                        h2r = []
                        for kb in range(KB):
                            t = pq.tile([128, OWN], BF, tag=f"h2r{kb}", bufs=2)
                            for t_ in range(OTB):
                                nc.sync.dma_start(
                                    t[:, t_ * 128:(t_ + 1) * 128],
                                    gth_out[t_][r * D + kb * 128: r * D + (kb + 1) * 128, :])
                            h2r.append(t)
                        hidp = [pq.tile([128, 2, OWN], F8, tag=f"hidp{p}", bufs=1,
                                        name=f"hidp{p}") for p in range(HB // 2)]
                        for h in range(HB):
                            ph = psC.tile([128, 512], FP, tag="ph", bufs=2)
                            for kb in range(KB):
                                nc.tensor.matmul(ph[:], ew1_sb[kb][:, h * 128:(h + 1) * 128],
                                                 h2r[kb][:], start=(kb == 0), stop=(kb == KB - 1))
                            nc.scalar.activation(hidp[h // 2][:, h % 2, :], ph[:],
                                                 AF.Gelu, bias=eb1_sb[:, h:h + 1])
                        # per-token weight for this core's expert
                        wvr = pq.tile([128, OTB, E], FP, tag="wvr", bufs=2)
                        nc.sync.dma_start(
                            wvr[:], gtw_out[r * OWN:(r + 1) * OWN, :]
                            .rearrange("(tb p) e -> p tb e", p=128))
                        ws = []
                        for tb in range(OTB):
                            wm_t = pq.tile([128, E], FP, tag="wm", bufs=2)
                            nc.vector.tensor_mul(wm_t[:], wvr[:, tb, :], esel[:])
                            ws_t = pq.tile([128, 1], FP, tag=f"ws{tb}", bufs=2)
                            nc.vector.tensor_reduce(ws_t[:], wm_t[:], mybir.AxisListType.X,
                                                    ALU.add)
                            ws.append(ws_t)
                        for nb in range(2):
                            peo = [psC.tile([128, 512], FP, tag=f"peo{t_}", bufs=1,
                                            name=f"peo{t_}") for t_ in range(OTB)]
                            for hc in range(4):
                                ew2c = pq.tile([128, 8, 512], F8, tag="ew2c", bufs=2)
                                for hh in range(8):
                                    nc.gpsimd.dma_start(
                                        ew2c[:, hh, :],
                                        dp["ew2"][(hc * 8 + hh) * 128:(hc * 8 + hh + 1) * 128,
                                                  nb * 512:(nb + 1) * 512])
                                for j in range(4):
                                    p = hc * 4 + j
                                    for tb in range(OTB):
                                        nc.tensor.matmul(
                                            peo[tb][:],
                                            hidp[p][:, :, tb * 128:(tb + 1) * 128],
                                            ew2c[:, 2 * j:2 * j + 2, :],
                                            start=(p == 0), stop=False, perf_mode=DR)
                            for tb in range(OTB):
                                nc.tensor.matmul(peo[tb][:], ones1b[:],
                                                 eb2h_sb[:, nb * 512:(nb + 1) * 512],
                                                 start=False, stop=True)
                                wout = pq.tile([128, 512], FP, tag="wout", bufs=3)
                                nc.vector.tensor_scalar(wout[:], peo[tb][:], ws[tb][:],
                                                        None, ALU.mult)
                                nc.vector.scalar_tensor_tensor(
                                    wout[:], xmid[tb][:, nb * 512:(nb + 1) * 512],
                                    rmask[:, r:r + 1], wout[:], ALU.mult, ALU.add)
                                nc.sync.dma_start(
                                    rs_in[r][tb * 128:(tb + 1) * 128,
                                             nb * 512:(nb + 1) * 512], wout[:])
                        nc.gpsimd.collective_compute(
                            "ReduceScatter", ALU.add, replica_groups=rg,
                            ins=[rs_in[r].opt()], outs=[rs_out[r].opt()])
                        nc.sync.dma_start(out_d[r * 128:(r + 1) * 128, :], rs_out[r][:])

    nc.compile()
    return nc


def host_prep(inputs):
    """Build the 8 per-core input maps from full inputs."""
    import ml_dtypes
    f32 = np.float32
    bf16 = ml_dtypes.bfloat16
    x = np.ascontiguousarray(np.asarray(inputs["x"], f32).reshape(B * T, D))
    n1 = np.asarray(inputs["norm1_w"], f32)
    n2 = np.asarray(inputs["norm2_w"], f32)
    ipw = np.ascontiguousarray(
        (np.asarray(inputs["in_proj_w"], f32) * n1[:, None]).astype(bf16))
    gw = np.ascontiguousarray(np.asarray(inputs["gate_w"], f32) * n2[:, None])
    import ml_dtypes as mld
    ew1f = np.asarray(inputs["e_w1"], f32) * n2[None, :, None]
    ew1b = ew1f.astype(bf16)
    ew2f = np.asarray(inputs["e_w2"], f32)
    # per-expert power-of-2 scale into fp8 e4m3 range (max ~240)
    s2 = np.array([2.0 ** np.floor(np.log2(192.0 / max(np.abs(ew2f[e]).max(), 1e-9)))
                   for e in range(E)], f32)
    ew2q = np.stack([(ew2f[e] * s2[e]).astype(mld.float8_e4m3) for e in range(E)])
    dbw = np.ascontiguousarray(np.concatenate(
        [np.asarray(inputs["dt_w"], f32), np.asarray(inputs["bp_w"], f32)], axis=1))
    ident = np.eye(128, dtype=f32)
    ones1 = np.ones((1, 128), f32)
    shared = {
        "ipw": ipw, "ipb": np.asarray(inputs["in_proj_b"], f32),
        "cw": np.ascontiguousarray(np.asarray(inputs["conv_w"], f32)[:, 0, :]),
        "cb": np.asarray(inputs["conv_b"], f32),
        "dbw": dbw,
        "dtb": np.asarray(inputs["dt_b"], f32), "bpb": np.asarray(inputs["bp_b"], f32),
        "cpw": np.asarray(inputs["cp_w"], f32), "cpb": np.asarray(inputs["cp_b"], f32),
        "s2iw": np.asarray(inputs["s2i_w"], f32).astype(bf16),
        "s2ib": np.asarray(inputs["s2i_b"], f32),
        "Dp": np.asarray(inputs["D_param"], f32),
        "ow": np.asarray(inputs["out_w"], f32).astype(bf16),
        "ob": np.asarray(inputs["out_b"], f32).astype(bf16),
        "gw": gw, "gb": np.asarray(inputs["gate_b"], f32),
        "ident": ident, "ones1": ones1, "ones1b": ones1.astype(bf16),
    }
    eb1 = np.asarray(inputs["e_b1"], f32)
    eb2 = np.asarray(inputs["e_b2"], f32)
    in_maps = []
    for c in range(N_CORES):
        e, th = c // 2, c % 2
        g0 = th * (B * T // 2) + e * OWN
        if e == 0:
            x_sh = np.concatenate([np.zeros((HALO, D), f32), x[g0:g0 + OWN]])
        else:
            x_sh = x[g0 - HALO:g0 + OWN]
        m = dict(shared)
        m["x_sh"] = np.ascontiguousarray(x_sh)
        m["ew1"] = np.ascontiguousarray(ew1b[e])
        m["eb1"] = np.ascontiguousarray(eb1[e])
        m["ew2"] = np.ascontiguousarray(ew2q[e])
        m["eb2h"] = np.ascontiguousarray((eb2[e] * s2[e]).astype(bf16))
        esel = np.zeros((128, E), f32)
        esel[:, e] = 1.0 / s2[e]  # dequant of fp8-scaled ew2 folded into combine weight
        m["esel"] = esel
        rmask = np.zeros((128, 4), f32)
        rmask[:, e] = 1.0
        m["rmask"] = rmask
        in_maps.append(m)
    return in_maps


def unshard_out(results):
    """results: list of 8 dicts with 'out' [OWN, D]; rows r*128+i of core c
    hold global token (c%2)*2048 + r*512 + (c//2)*128 + i."""
    full = np.empty((B * T, D), np.float32)
    for c in range(N_CORES):
        e, th = c // 2, c % 2
        oc = results[c]["out"]
        for r in range(4):
            full[th * 2048 + r * OWN + e * 128: th * 2048 + r * OWN + (e + 1) * 128] = \
                oc[r * 128:(r + 1) * 128]
    return full.reshape(B, T, D)


_NC_CACHE = {}


def _get_nc():
    if "nc" not in _NC_CACHE:
        _NC_CACHE["nc"] = build(debug_outputs=False)
    return _NC_CACHE["nc"]


def kernel(**inputs) -> np.ndarray:
    """Full-input entry point: shards across 8 NeuronCores, runs the Bass
    kernel SPMD, reassembles the full [2, 2048, 1024] output."""
    import sys, types
    try:  # NTFF profile hook shim (missing antenv.axon_hooks in this image)
        import antenv.axon_hooks  # noqa: F401
    except ImportError:
        try:
            import antenv
            from trn_agent_boot.trn_boot import _ntff_profile_via_ctypes
            mod = types.ModuleType("antenv.axon_hooks")
            try:
                _hook = _ntff_profile_via_ctypes("/opt/axon/libaxon_pjrt.so")
            except Exception:
                _hook = None
            mod.get_axon_ntff_profile_hook = lambda: _hook
            mod.set_axon_ntff_profile_hook = lambda h: None
            sys.modules["antenv.axon_hooks"] = mod
            antenv.axon_hooks = mod
        except Exception:
            pass
    from concourse.bass_utils import run_bass_kernel_spmd

    nc = _get_nc()
    in_maps = host_prep(inputs)
    res = run_bass_kernel_spmd(nc, in_maps, core_ids=list(range(N_CORES)))
    out = unshard_out(res.results)
    return out.astype(np.float32)
